# revision 2
# baseline (speedup 1.0000x reference)
"""Trainium2 Bass kernel for nn_CrossAttention (dense_transformer).

Sharding: data-parallel over batch B=8 across 8 NeuronCores (1 sample
per core). BatchNorm uses batch statistics, so per-channel partial
[sum, sumsq] are all-reduced across cores ([128,2] f32 payload, 2x).

Per-core layout: activations [C=128 partitions, N=H*W=2304 free]. All
heavy matmuls run in float32r (TF32-like, 1 PE cycle/row at free>=256,
4x fp32). The attention value path (exp output, v^T, softmax partial
sums) runs in bf16.

Attention in energy-transposed layout with algebraically folded
projections (saves the k-projection and all per-block transposes):
  energy^T[m,q] = sum_c yp[c,m] * qh[c,q],  qh = (Wq^T Wk)^T @ h
  v^T[mo]       = matmul(lhsT=yp[:,mo-chunk], rhs=gamma*Wv^T)
Softmax reduces over m = partitions (no max-subtraction: |energy| <=
~10): per-m-chunk exp'd energies are pair-summed + tree-folded on
DVE/GPSIMD, one ones-matmul per q-superblock does the partition sum
(arrives pre-broadcast); 1/colsum applied after the v-contraction.

Scheduling: input DMA split into ~64KB pieces in consumption order;
projections/vT/qh are emitted per-chunk interleaved into the first
q-superblock of attention 1, so the PE starts ~3us in. Attention runs
per m-chunk (18 steps/qsb): energy matmuls 2 steps ahead into a 4-deep
single-bank PSUM ring, one Exp per chunk (ACT is the rate limiter at
~530ns/chunk), attn-out accumulator double-buffered so the qsb
epilogue (colsum matmul -> recip -> mult -> residual) overlaps the
next qsb's steps. conv3x3 = 9 shifted-window matmuls over a
zero-padded [128,50,50] buffer.
"""

import sys

sys.path.insert(0, "/opt/trn_rl_repo")

import numpy as np

_NC_CACHE = {}

B, CIN, C, H, W = 8, 256, 128, 48, 48
N = H * W  # 2304
P = 128
NKO = CIN // P  # 2
NMO = N // P  # 18
# q superblocks: row-aligned chunks (48-col rows); 480 = 10 rows
QCH = [(0, 480), (480, 480), (960, 480), (1440, 480), (1920, 384)]
ROWCH = [(0, 10), (10, 10), (20, 10), (30, 10), (40, 8)]
# vT[mo] becomes computable once yp chunks covering cols [128mo,128mo+128)
# are projected; chunk c covers cols [480c, 480c+480)
VT_BY_CHUNK = [[0, 1, 2], [3, 4, 5, 6], [7, 8, 9, 10], [11, 12, 13, 14], [15, 16, 17]]
NSTAT = float(B * N)  # BN stat count over (B,H,W)
EPS = 1e-5


def _build(variant="default"):
    """variant: 'default' = 8-core w/ collectives; 'sim' = single-core,
    collectives replaced by DMA copy (for TimelineSim profiling)."""
    key = f"nc_{variant}"
    if key in _NC_CACHE:
        return _NC_CACHE[key]

    import concourse.mybir as mybir
    import concourse.tile as tile
    from concourse import bacc
    from contextlib import ExitStack

    F32 = mybir.dt.float32
    F32R = mybir.dt.float32r
    BF16 = mybir.dt.bfloat16
    AF = mybir.ActivationFunctionType
    ALU = mybir.AluOpType
    AX = mybir.AxisListType

    sim = variant == "sim"
    nc = bacc.Bacc(
        "TRN2", target_bir_lowering=False, debug=False,
        num_devices=1 if sim else 8,
    )

    # ---- DRAM I/O (f32r tensors carry plain fp32 bits; PE rounds) ----
    d_x = nc.dram_tensor("x", [CIN, N], F32R, kind="ExternalInput")
    d_y = nc.dram_tensor("y", [CIN, N], F32R, kind="ExternalInput")
    d_w_inT = nc.dram_tensor("w_inT", [P, NKO, P], F32R, kind="ExternalInput")
    d_b_in = nc.dram_tensor("b_in", [P, 1], F32, kind="ExternalInput")
    d_A1 = nc.dram_tensor("A1", [P, P], F32R, kind="ExternalInput")
    d_gwv1T = nc.dram_tensor("gwv1T", [P, 256], F32R, kind="ExternalInput")
    d_A2 = nc.dram_tensor("A2", [P, P], F32R, kind="ExternalInput")
    d_gwv2T = nc.dram_tensor("gwv2T", [P, 256], F32R, kind="ExternalInput")
    d_w1T = nc.dram_tensor("w1T", [P, 9, P], F32R, kind="ExternalInput")
    d_bn1s = nc.dram_tensor("bn1s", [P, 1], F32, kind="ExternalInput")
    d_bn1b = nc.dram_tensor("bn1b", [P, 1], F32, kind="ExternalInput")
    d_w2T = nc.dram_tensor("w2T", [P, 9, P], F32R, kind="ExternalInput")
    d_bn2s = nc.dram_tensor("bn2s", [P, 1], F32, kind="ExternalInput")
    d_bn2b = nc.dram_tensor("bn2b", [P, 1], F32, kind="ExternalInput")
    d_predT = nc.dram_tensor("predT", [P, P], F32R, kind="ExternalInput")
    d_pred_b = nc.dram_tensor("pred_b", [1, 1], F32, kind="ExternalInput")
    d_out = nc.dram_tensor("out", [1, N], F32, kind="ExternalOutput")

    with tile.TileContext(nc) as tc, ExitStack() as ctx:
        wgt = ctx.enter_context(tc.tile_pool(name="wgt", bufs=1))
        act = ctx.enter_context(tc.tile_pool(name="act", bufs=1))
        ew = ctx.enter_context(tc.tile_pool(name="ew", bufs=1))
        eeP = ctx.enter_context(tc.tile_pool(name="eeP", bufs=8))
        load = ctx.enter_context(tc.tile_pool(name="load", bufs=4))
        dram = ctx.enter_context(tc.tile_pool(name="dram", bufs=1, space="DRAM"))
        # PSUM: pE 4x[128,512] + pO 2x[128,480] + pM 2x[128,480] = 8 banks
        pE = ctx.enter_context(tc.tile_pool(name="pE", bufs=4, space="PSUM"))
        pO = ctx.enter_context(tc.tile_pool(name="pO", bufs=2, space="PSUM"))
        pM = ctx.enter_context(tc.tile_pool(name="pM", bufs=2, space="PSUM"))

        # ---------- weights (direct DMA; PE rounds f32r on read) ----------
        def load_w(dsrc, shape, tag, dtype=F32R):
            t = wgt.tile(shape, dtype, tag=tag)
            nc.sync.dma_start(t[:], dsrc[...])
            return t

        w_inT_r = load_w(d_w_inT, [P, NKO, P], "w_inT_r")
        A1_r = load_w(d_A1, [P, P], "A1_r")
        gwv1T_r = load_w(d_gwv1T, [P, 256], "gwv1T_r")
        A2_r = load_w(d_A2, [P, P], "A2_r")
        gwv2T_r = load_w(d_gwv2T, [P, 256], "gwv2T_r")
        w1T_r = load_w(d_w1T, [P, 9, P], "w1T_r")
        w2T_r = load_w(d_w2T, [P, 9, P], "w2T_r")
        predT_r = load_w(d_predT, [P, P], "predT_r")
        b_in = load_w(d_b_in, [P, 1], "b_in", F32)
        bn1s = load_w(d_bn1s, [P, 1], "bn1s", F32)
        bn1b = load_w(d_bn1b, [P, 1], "bn1b", F32)
        bn2s = load_w(d_bn2s, [P, 1], "bn2s", F32)
        bn2b = load_w(d_bn2b, [P, 1], "bn2b", F32)
        pred_b = load_w(d_pred_b, [1, 1], "pred_b", F32)

        ones_f = wgt.tile([P, P], F32, tag="ones_f")
        nc.gpsimd.memset(ones_f[:], 1.0)
        ones_b = wgt.tile([P, P], BF16, tag="ones_b")
        nc.vector.tensor_copy(ones_b[:], ones_f[:])

        zrow = wgt.tile([P, W + 2], F32, tag="zrow")
        nc.gpsimd.memset(zrow[:], 0.0)

        # ---------- stage A: input DMA, split small and ordered so the
        # first-needed data lands in ~3us; proj/vT/qh emitted per-chunk
        # (chunk 0 as prelude, chunks 1-4 as side-work inside attention 1)
        xr = [load.tile([P, N], F32R, tag="in_r", name=f"xr{k}") for k in range(NKO)]
        yr = [load.tile([P, N], F32R, tag="in_r", name=f"yr{k}") for k in range(NKO)]
        xp = act.tile([P, N], F32R, tag="tagA")
        yp = act.tile([P, N], F32R, tag="tagB")

        def emit_load(dsrc, rr, c, parts):
            q0, qn = QCH[c]
            sub = qn // parts
            for s in range(parts):
                for ko in range(NKO):
                    c0 = q0 + s * sub
                    nc.sync.dma_start(
                        rr[ko][:, c0 : c0 + sub],
                        dsrc[ko * P : (ko + 1) * P, c0 : c0 + sub],
                    )

        emit_load(d_y, yr, 0, 4)
        emit_load(d_x, xr, 0, 4)
        for c in range(1, 5):
            emit_load(d_y, yr, c, 2)
        for c in range(1, 5):
            emit_load(d_x, xr, c, 2)

        def proj_chunk(rr, dst, c):
            q0, qn = QCH[c]
            ps = pM.tile([P, 480], F32, tag="mps")
            for ko in range(NKO):
                nc.tensor.matmul(
                    ps[:, :qn], w_inT_r[:, ko, :], rr[ko][:, q0 : q0 + qn],
                    start=(ko == 0), stop=(ko == NKO - 1),
                )
            nc.vector.tensor_scalar_add(dst[:, q0 : q0 + qn], ps[:, :qn], b_in[:])

        def build_vT_chunk(vT, gwvT_r, mos):
            # vT[mo][m, c] = sum_c' yp[c', mo*P+m] * (gamma*wv^T)[c', c]
            # rhs zero-padded to 256 cols so f32r streams at full rate
            for mo in mos:
                pst = pM.tile([P, 480], F32, tag="mps")
                nc.tensor.matmul(
                    pst[:, :256], yp[:, mo * P : (mo + 1) * P], gwvT_r[:],
                    start=True, stop=True,
                )
                nc.vector.tensor_copy(vT[:, mo, :], pst[:, :P])

        def qh_chunk(A_r, src_r, dst, c):
            q0, qn = QCH[c]
            ps = pM.tile([P, 480], F32, tag="mps")
            nc.tensor.matmul(
                ps[:, :qn], A_r[:], src_r[:, q0 : q0 + qn], start=True, stop=True
            )
            nc.vector.tensor_copy(dst[:, q0 : q0 + qn], ps[:, :qn])

        # ---------- helpers ----------
        def zero_pad_border(pad):
            nc.vector.tensor_copy(pad[:, 0, :], zrow[:])
            nc.vector.tensor_copy(pad[:, H + 1, :], zrow[:])
            nc.vector.tensor_copy(pad[:, 1 : H + 1, 0:1], zrow[:, :H, None])
            nc.vector.tensor_copy(pad[:, 1 : H + 1, W + 1 : W + 2], zrow[:, :H, None])

        def attention(qh_r, vT_r, resid_r, pad_tag, side_work=None):
            pad = act.tile([P, H + 2, W + 2], F32R, tag=pad_tag)
            zero_pad_border(pad)
            steps = [(qi, mo) for qi in range(len(QCH)) for mo in range(NMO)]
            AHEAD = 2

            def emit_energy(qi, mo):
                q0, qn = QCH[qi]
                ps_e = pE.tile([P, 512], F32, tag="energy")
                nc.tensor.matmul(
                    ps_e[:, :qn],
                    yp[:, mo * P : (mo + 1) * P],
                    qh_r[:, q0 : q0 + qn],
                    start=True, stop=True,
                )
                return ps_e

            pend = {}
            for k in range(AHEAD):
                pend[k] = emit_energy(*steps[k])
            ps_o = None
            prs = []
            ee_prev = None
            for idx, (qi, mo) in enumerate(steps):
                q0, qn = QCH[qi]
                if idx + AHEAD < len(steps):
                    pend[idx + AHEAD] = emit_energy(*steps[idx + AHEAD])
                ps_e = pend.pop(idx)
                if mo == 0:
                    ps_o = pO.tile([P, 480], F32, tag="attn_out")
                    prs = []
                ee = eeP.tile([P, 512], BF16, tag="ee")
                nc.scalar.activation(ee[:, :qn], ps_e[:, :qn], AF.Exp)
                nc.tensor.matmul(
                    ps_o[:, :qn], vT_r[:, mo, :], ee[:, :qn],
                    start=(mo == 0), stop=(mo == NMO - 1),
                )
                if mo % 2 == 0:
                    ee_prev = ee
                else:
                    # pair-sum + incremental binary-counter fold (<=4 live)
                    j = mo // 2
                    pr = eeP.tile([P, 512], BF16, tag="pair")
                    eng = nc.gpsimd if j % 2 == 0 else nc.vector
                    eng.tensor_tensor(
                        pr[:, :qn], ee_prev[:, :qn], ee[:, :qn], ALU.add
                    )
                    lv, t = 0, pr
                    while prs and prs[-1][0] == lv:
                        prev = prs.pop()[1]
                        o = eeP.tile([P, 512], BF16, tag="fold")
                        nc.vector.tensor_tensor(
                            o[:, :qn], prev[:, :qn], t[:, :qn], ALU.add
                        )
                        t, lv = o, lv + 1
                    prs.append((lv, t))
                if mo == NMO - 1:
                    while len(prs) > 1:
                        (_, a), (_, b2) = prs.pop(), prs.pop()
                        o = eeP.tile([P, 512], BF16, tag="fold")
                        nc.vector.tensor_tensor(
                            o[:, :qn], a[:, :qn], b2[:, :qn], ALU.add
                        )
                        prs.append((99, o))
                    # colsum via one ones-matmul, borrowing an energy slot;
                    # result arrives pre-broadcast across partitions
                    ps_s = pE.tile([P, 512], F32, tag="energy")
                    nc.tensor.matmul(
                        ps_s[:, :qn], ones_b[:], prs.pop()[1][:, :qn],
                        start=True, stop=True,
                    )
                    rcp = ew.tile([P, 480], F32, tag="recip")
                    nc.vector.reciprocal_approx_fast(rcp[:, :qn], ps_s[:, :qn])
                    tmp = ew.tile([P, 480], F32, tag="tmp")
                    nc.vector.tensor_tensor(
                        tmp[:, :qn], ps_o[:, :qn], rcp[:, :qn], ALU.mult
                    )
                    r0, nr = q0 // W, qn // W
                    nc.vector.tensor_tensor(
                        pad[:, 1 + r0 : 1 + r0 + nr, 1 : W + 1],
                        tmp[:, :qn].rearrange("p (a b) -> p a b", b=W),
                        resid_r[:, q0 : q0 + qn].rearrange("p (a b) -> p a b", b=W),
                        ALU.add,
                    )
                if side_work is not None and idx in side_work:
                    side_work[idx]()
            return pad

        def conv_bn_relu(pad, wT_r, bns, bnb, t_tag, out_tag, ar_idx, overlap_fn=None):
            # conv3x3 SAME via 9 shifted-window matmuls; batch-stat allreduce
            t_sb = act.tile([P, N], F32, tag=t_tag)
            sums = ew.tile([P, len(ROWCH)], F32, tag="sums")
            sqs = ew.tile([P, len(ROWCH)], F32, tag="sqs")
            for ci, (r0, nr) in enumerate(ROWCH):
                qn = nr * W
                ps = pM.tile([P, 480], F32, tag="mps")
                t = 0
                for dy in range(3):
                    for dx in range(3):
                        nc.tensor.matmul(
                            ps[:, :qn],
                            wT_r[:, t, :],
                            pad[:, dy + r0 : dy + r0 + nr, dx : dx + W],
                            start=(t == 0),
                            stop=(t == 8),
                        )
                        t += 1
                q0 = r0 * W
                nc.vector.tensor_copy(t_sb[:, q0 : q0 + qn], ps[:, :qn])
                nc.vector.reduce_sum(sums[:, ci : ci + 1], ps[:, :qn], axis=AX.X)
                scr = ew.tile([P, 480], F32, tag="sq_scr")
                nc.scalar.activation(
                    scr[:, :qn], ps[:, :qn], AF.Square,
                    accum_out=sqs[:, ci : ci + 1],
                )
            stats = ew.tile([P, 2], F32, tag="stats")
            nc.vector.reduce_sum(stats[:, 0:1], sums[:], axis=AX.X)
            nc.vector.reduce_sum(stats[:, 1:2], sqs[:], axis=AX.X)
            nc.vector.tensor_scalar_mul(stats[:], stats[:], 1.0 / NSTAT)
            # fold +EPS into the allreduced sumsq (each core adds EPS/8)
            nc.vector.tensor_scalar_add(stats[:, 1:2], stats[:, 1:2], EPS / 8.0)
            cc_in = dram.tile([P, 2], F32, tag=f"cc_in{ar_idx}")
            cc_out = dram.tile([P, 2], F32, tag=f"cc_out{ar_idx}")
            nc.sync.dma_start(cc_in[:], stats[:])
            if sim:
                nc.sync.dma_start(cc_out[:], cc_in[:])
            else:
                nc.gpsimd.collective_compute(
                    "AllReduce",
                    ALU.add,
                    replica_groups=[list(range(8))],
                    ins=[cc_in[:].opt()],
                    outs=[cc_out[:].opt()],
                )
            if overlap_fn is not None:
                overlap_fn()
            st_all = ew.tile([P, 2], F32, tag="st_all")
            nc.sync.dma_start(st_all[:], cc_out[:])
            mean = st_all[:, 0:1]
            var = ew.tile([P, 1], F32, tag="var")
            nc.vector.tensor_tensor(var[:], mean, mean, ALU.mult)
            # var+eps = m2e - mean^2, fused: (var * -1 + m2e)
            nc.vector.scalar_tensor_tensor(
                var[:], var[:], -1.0, st_all[:, 1:2], ALU.mult, ALU.add
            )
            std = ew.tile([P, 1], F32, tag="std")
            nc.scalar.activation(std[:], var[:], AF.Sqrt)
            a_sc = ew.tile([P, 1], F32, tag="a_sc")
            with nc.allow_low_precision(reason="bn rsqrt"):
                nc.vector.reciprocal(a_sc[:], std[:])
            nc.vector.tensor_tensor(a_sc[:], a_sc[:], bns[:], ALU.mult)
            c_bi = ew.tile([P, 1], F32, tag="c_bi")
            nc.vector.tensor_tensor(c_bi[:], mean, a_sc[:], ALU.mult)
            nc.vector.tensor_tensor(c_bi[:], bnb[:], c_bi[:], ALU.subtract)
            h_out = act.tile([P, N], F32R, tag=out_tag)
            for q0, qn in QCH:
                nc.scalar.activation(
                    h_out[:, q0 : q0 + qn], t_sb[:, q0 : q0 + qn],
                    AF.Relu, bias=c_bi[:], scale=a_sc[:],
                )
            return h_out

        # ---------- pipeline ----------
        qh1 = act.tile([P, N], F32R, tag="tagC")
        vT1 = act.tile([P, NMO, P], BF16, tag="vT1")
        # prelude: chunk-0 projections so attention 1 can start immediately
        proj_chunk(yr, yp, 0)
        build_vT_chunk(vT1, gwv1T_r, VT_BY_CHUNK[0])
        proj_chunk(xr, xp, 0)
        qh_chunk(A1_r, xp, qh1, 0)

        def mk_side1(c):
            def f():
                proj_chunk(yr, yp, c)
                build_vT_chunk(vT1, gwv1T_r, VT_BY_CHUNK[c])
                proj_chunk(xr, xp, c)
                qh_chunk(A1_r, xp, qh1, c)
            return f

        side1 = {2 * (c - 1): mk_side1(c) for c in range(1, 5)}
        h1pad = attention(qh1, vT1, xp, "tagE", side_work=side1)

        vT2 = act.tile([P, NMO, P], BF16, tag="vT2")
        h2 = conv_bn_relu(
            h1pad, w1T_r, bn1s, bn1b, "tagT", "h2", 1,
            overlap_fn=lambda: build_vT_chunk(vT2, gwv2T_r, range(NMO)),
        )
        qh2 = act.tile([P, N], F32R, tag="tagC")
        qh_chunk(A2_r, h2, qh2, 0)

        def mk_side2(c):
            def f():
                qh_chunk(A2_r, h2, qh2, c)
            return f

        side2 = {18 * c - 12: mk_side2(c) for c in range(1, 5)}
        h3pad = attention(qh2, vT2, h2, "tagE", side_work=side2)
        r2 = conv_bn_relu(h3pad, w2T_r, bn2s, bn2b, "tagT", "h2", 2)

        # ---------- pred head ----------
        out_sb = act.tile([1, N], F32, tag="out_sb")
        for q0, qn in QCH:
            ps = pM.tile([P, 480], F32, tag="mps")
            nc.tensor.matmul(
                ps[:, :qn], predT_r[:], r2[:, q0 : q0 + qn], start=True, stop=True
            )
            nc.vector.tensor_scalar_add(
                out_sb[:, q0 : q0 + qn], ps[0:1, :qn], pred_b[:]
            )
        nc.sync.dma_start(d_out[:, :], out_sb[:])

    nc.compile()
    _NC_CACHE[key] = nc
    return nc


def _install_ntff_hook():
    """Register the axon NTFF profiling hook (antenv.axon_hooks is absent
    in this image; libaxon_pjrt.so exports the C ABI — same wiring as
    trn_agent_boot's _ntff_profile_via_ctypes)."""
    import sys as _sys, types, ctypes, contextlib

    if "antenv.axon_hooks" in _sys.modules:
        return
    try:
        lib = ctypes.CDLL("/opt/axon/libaxon_pjrt.so")
        lib.axon_start_nrt_profile.argtypes = [
            ctypes.POINTER(ctypes.c_int64), ctypes.c_size_t,
        ]
        lib.axon_start_nrt_profile.restype = ctypes.c_int64
        lib.axon_stop_nrt_profile.argtypes = [ctypes.c_char_p]
        lib.axon_stop_nrt_profile.restype = ctypes.c_int64
    except (OSError, AttributeError):
        return

    @contextlib.contextmanager
    def _hook(output_dir, device_ids):
        import jax

        jax.devices()
        if device_ids:
            ids = (ctypes.c_int64 * len(device_ids))(*device_ids)
            rc = lib.axon_start_nrt_profile(ids, len(device_ids))
        else:
            rc = lib.axon_start_nrt_profile(None, 0)
        if rc != 0:
            raise RuntimeError(f"axon_start_nrt_profile rc={rc}")
        try:
            yield
        finally:
            n = lib.axon_stop_nrt_profile(str(output_dir).encode())
            if n < 0:
                raise RuntimeError(f"axon_stop_nrt_profile rc={n}")

    mod = types.ModuleType("antenv.axon_hooks")
    mod.get_axon_ntff_profile_hook = lambda: _hook
    mod.set_axon_ntff_profile_hook = lambda h: None
    _sys.modules["antenv.axon_hooks"] = mod
    # artifact upload has no bucket in this container; keep files local
    import concourse.bass_utils as _bu

    _bu.upload_artifacts = lambda d: d


def kernel(**inputs):
    from concourse.bass_utils import run_bass_kernel_spmd
    import os

    nc = _build()

    f32 = np.float32
    x = np.ascontiguousarray(inputs["x"], dtype=f32).reshape(B, CIN, N)
    y = np.ascontiguousarray(inputs["y"], dtype=f32).reshape(B, CIN, N)
    w_in = np.asarray(inputs["w_in"], dtype=f32)
    b_in = np.asarray(inputs["b_in"], dtype=f32).reshape(P, 1)
    ca_wq = np.asarray(inputs["ca_wq"], dtype=f32)
    ca_wk = np.asarray(inputs["ca_wk"], dtype=f32)
    ca_wv = np.asarray(inputs["ca_wv"], dtype=f32)
    g1 = np.asarray(inputs["ca_gamma"], dtype=f32).reshape(-1)[0]
    sa_wq = np.asarray(inputs["sa_wq"], dtype=f32)
    sa_wk = np.asarray(inputs["sa_wk"], dtype=f32)
    sa_wv = np.asarray(inputs["sa_wv"], dtype=f32)
    g2 = np.asarray(inputs["sa_gamma"], dtype=f32).reshape(-1)[0]
    conv1_w = np.asarray(inputs["conv1_w"], dtype=f32)
    conv2_w = np.asarray(inputs["conv2_w"], dtype=f32)
    bn1s = np.asarray(inputs["bn1_s"], dtype=f32).reshape(P, 1)
    bn1b = np.asarray(inputs["bn1_b"], dtype=f32).reshape(P, 1)
    bn2s = np.asarray(inputs["bn2_s"], dtype=f32).reshape(P, 1)
    bn2b = np.asarray(inputs["bn2_b"], dtype=f32).reshape(P, 1)
    pred_w = np.asarray(inputs["pred_w"], dtype=f32)
    pred_b = np.asarray(inputs["pred_b"], dtype=f32).reshape(1, 1)

    # host-side weight prep (small, O(C^2))
    w_inT = np.ascontiguousarray(
        w_in.T.reshape(NKO, P, P).transpose(1, 0, 2)
    )  # [cin_p, ko, cout]
    A1 = np.ascontiguousarray(ca_wq.T @ ca_wk)
    A2 = np.ascontiguousarray(sa_wq.T @ sa_wk)
    # gamma*Wv^T zero-padded to 256 cols (f32r full-rate moving operand)
    gwv1T = np.zeros((P, 256), f32)
    gwv1T[:, :P] = g1 * ca_wv.T
    gwv2T = np.zeros((P, 256), f32)
    gwv2T[:, :P] = g2 * sa_wv.T
    # conv taps: [o, i, 3, 3] -> lhsT per tap [i, o]; layout [i_p, tap, o]
    w1T = np.ascontiguousarray(
        conv1_w.transpose(2, 3, 1, 0).reshape(9, P, P).transpose(1, 0, 2)
    )
    w2T = np.ascontiguousarray(
        conv2_w.transpose(2, 3, 1, 0).reshape(9, P, P).transpose(1, 0, 2)
    )
    predT = np.zeros((P, P), f32)
    predT[:, 0] = pred_w[0]

    shared = {
        "w_inT": w_inT, "b_in": b_in, "A1": A1, "gwv1T": gwv1T,
        "A2": A2, "gwv2T": gwv2T,
        "w1T": w1T, "bn1s": bn1s,
        "bn1b": bn1b, "w2T": w2T, "bn2s": bn2s, "bn2b": bn2b,
        "predT": predT, "pred_b": pred_b,
    }
    in_maps = [
        {"x": np.ascontiguousarray(x[i]), "y": np.ascontiguousarray(y[i]), **shared}
        for i in range(B)
    ]

    trace = bool(int(os.environ.get("KERNEL_TRACE", "0")))
    if trace:
        _install_ntff_hook()
    res = run_bass_kernel_spmd(nc, in_maps, core_ids=list(range(B)), trace=trace)
    if trace:
        _NC_CACHE["last_results"] = res
    out = np.stack(
        [res.results[i]["out"].reshape(1, H, W) for i in range(B)]
    ).astype(f32)
    return out
